# revision 28
# baseline (speedup 1.0000x reference)
"""GQA attention kernel for Trainium2, tensor-parallel over heads on 8 cores.

Problem: B=1, T=2048, EMB=4096, H=32 query heads, G=8 KV groups, D=128.
Reference: q/k/v projections -> per-head RMS norm (q,k) -> RoPE (q,k) ->
causal GQA attention -> out projection.

Sharding: core c owns query heads [4c, 4c+4) and KV group c.  Each core
computes a partial output for its heads; host sums the 8 partials (the
all-reduce of the module's TP scheme, done on host since full I/O is
required anyway).

v5 (450us v4 -> target ~400us): xstrip/cs prefetch depth 3 (strip-boundary
PE micro-gaps were resetting the PE p-state); strips 0+1 share one e-loop
to halve the startup weight-download stall; rms stats batched into one
sqrt+reciprocal per strip; rope for head 3 + k offloaded to the idle
GpSimd(Pool) engine; per-block denominator matmuls replaced by DVE
accumulation of exp mass + one f32r broadcast matmul per group.

v4 (482us v3 -> target ~430us): psA bufs=4 + lookahead-3 S pipeline;
diagonal blocks narrowed to their unmasked query columns in
S/mask/exp/den/ctx; phase C copies rebalanced 5:3 ACT:DVE; opool bufs=3.

v3 (487us v2 -> target ~420us):
  - Phase B flattened: one software pipeline per 512-query slice across
    all 4 heads (S matmuls 2 blocks ahead), si-outer loop; the group
    epilogue (reciprocal+normalize) lands on the DVE queue after the
    next group's mask add, off the critical path.
  - Phase C interleaved per si-group right after its 4 head-groups:
    output DMA drains during attention instead of all at the end.
  - reciprocal_approx_fast (5x faster DVE reciprocal, 18-bit) for both
    softmax denominators and rms rstd.
  - Phase C PSUM->SBUF copies alternate between ACT and DVE.
  - Weight DMAs split into ~512KB posts interleaved in e-order so the
    first projection matmuls start as soon as their slice arrives.
"""

import numpy as np
import ml_dtypes
from contextlib import ExitStack

import concourse.bass as bass
import concourse.bacc as bacc
import concourse.mybir as mybir
from concourse.tile import TileContext
from concourse.bass_utils import run_bass_kernel_spmd
from concourse.masks import make_identity

EMB, H, G, D, T = 4096, 32, 8, 128, 2048
EPS = 1e-6
NCORES = 8
HP = H // NCORES          # 4 query heads per core
NT = T // 128             # 16 t-tiles
NE = EMB // 128           # 32 e-tiles
NO = EMB // 512           # 8 output column tiles
QW = HP * D               # 512 = q width per core
KVW = 2 * D               # 256 = k|v width per core
SM_SCALE = 1.0 / float(np.sqrt(D))
NEG = -1e9

F32 = mybir.dt.float32
BF16 = mybir.dt.bfloat16
BF = ml_dtypes.bfloat16

_prog_cache = {}


def _build_program():
    nc = bacc.Bacc()

    xT_d = nc.declare_dram_parameter("xT", [NT * 128, NE * 128], BF16, isOutput=False)
    wq_d = nc.declare_dram_parameter("wq", [128, NE * QW], BF16, isOutput=False)
    wkv_d = nc.declare_dram_parameter("wkv", [128, NE * KVW], BF16, isOutput=False)
    wo_d = nc.declare_dram_parameter("wo", [128, HP * EMB], BF16, isOutput=False)
    # [cosq x4 | sinq x4 | cosk | sink]; q tables tiled 4-wide to match the
    # half-split head-interleaved q layout (all first-halves, then second)
    cs_d = nc.declare_dram_parameter("cs", [NT * 128, 1280], F32, isOutput=False)
    mask_d = nc.declare_dram_parameter("maskT", [128, 896], F32, isOutput=False)
    bias_d = nc.declare_dram_parameter("biasb", [128, QW + KVW], F32, isOutput=False)
    out_d = nc.declare_dram_parameter("out", [T, EMB], BF16, isOutput=True)

    with TileContext(nc) as tc, ExitStack() as ctx:
        consts = ctx.enter_context(tc.tile_pool(name="consts", bufs=1))
        wpool = ctx.enter_context(tc.tile_pool(name="wpool", bufs=1))
        xpool = ctx.enter_context(tc.tile_pool(name="xpool", bufs=3))
        cspool = ctx.enter_context(tc.tile_pool(name="cspool", bufs=2))
        scratch = ctx.enter_context(tc.tile_pool(name="scratch", bufs=3))
        small = ctx.enter_context(tc.tile_pool(name="small", bufs=4))
        ppool = ctx.enter_context(tc.tile_pool(name="ppool", bufs=4))
        epool = ctx.enter_context(tc.tile_pool(name="epool", bufs=2))
        opool = ctx.enter_context(tc.tile_pool(name="opool", bufs=2))
        resid = ctx.enter_context(tc.tile_pool(name="resid", bufs=1))
        psA = ctx.enter_context(tc.tile_pool(name="psA", bufs=4, space="PSUM"))
        psB = ctx.enter_context(tc.tile_pool(name="psB", bufs=2, space="PSUM"))
        psC = ctx.enter_context(tc.tile_pool(name="psC", bufs=2, space="PSUM"))

        # engine-side constants (no DMA involved)
        ident = consts.tile([128, 128], BF16, tag="ident", name="ident")
        make_identity(nc, ident)
        ones_f32 = consts.tile([128, 128], F32, tag="ones_f32", name="ones_f32")
        nc.vector.memset(ones_f32, 1.0)
        eps_t = consts.tile([128, 1], F32, tag="eps", name="eps")
        nc.vector.memset(eps_t, EPS)

        # strips 0/1 inputs first so phase A can start while weights stream in
        xstrips01 = []
        css01 = []
        for it in range(2):
            xs = xpool.tile([128, NE * 128], BF16, tag="xstrip", name=f"xstrip{it}")
            r0, r1 = it * 128, (it + 1) * 128
            nc.sync.dma_start(out=xs[:, 0:2048], in_=xT_d[r0:r1, 0:2048])
            nc.sync.dma_start(out=xs[:, 2048:4096], in_=xT_d[r0:r1, 2048:4096])
            cst = cspool.tile([128, 1280], F32, tag="cs", name=f"cs{it}")
            nc.sync.dma_start(out=cst, in_=cs_d[r0:r1, :])
            xstrips01.append(xs)
            css01.append(cst)
        bias_sb = consts.tile([128, QW + KVW], F32, tag="bias", name="bias")
        nc.sync.dma_start(out=bias_sb, in_=bias_d[:, :])

        # resident weights, posted in e-order in ~512KB chunks
        wq_sb = wpool.tile([128, NE * QW], BF16, tag="wq", name="wq")
        wkv_sb = wpool.tile([128, NE * KVW], BF16, tag="wkv", name="wkv")
        for ch in range(8):
            c0, c1 = ch * 4 * QW, (ch + 1) * 4 * QW
            nc.sync.dma_start(out=wq_sb[:, c0:c1], in_=wq_d[:, c0:c1])
            if ch % 2 == 0:
                k0, k1 = ch * 4 * KVW, (ch + 2) * 4 * KVW
                nc.sync.dma_start(out=wkv_sb[:, k0:k1], in_=wkv_d[:, k0:k1])
        mask_sb = consts.tile([128, 896], F32, tag="mask", name="mask")
        nc.sync.dma_start(out=mask_sb, in_=mask_d[:, :])

        # out-proj weights last (not needed until phase C)
        wo_sb = wpool.tile([128, HP * EMB], BF16, tag="wo", name="wo")
        nc.sync.dma_start(out=wo_sb[:, 0:HP * EMB // 2], in_=wo_d[:, 0:HP * EMB // 2])
        nc.sync.dma_start(out=wo_sb[:, HP * EMB // 2:], in_=wo_d[:, HP * EMB // 2:])

        # resident activations
        qT = [resid.tile([128, T], BF16, tag=f"qT{h}", name=f"qT{h}") for h in range(HP)]
        kT = resid.tile([128, T], BF16, tag="kT", name="kT")
        vsb = [resid.tile([128, 128], BF16, tag=f"v{j}", name=f"v{j}") for j in range(NT)]
        ctxT = [resid.tile([128, T], BF16, tag=f"ctxT{h}", name=f"ctxT{h}") for h in range(HP)]

        # ---------------- Phase A: projections + rms + rope + transpose ----
        # q layout is half-split head-interleaved: cols [h*64+d'] hold head
        # h dims 0..63, cols [256+h*64+d'] hold dims 64..127.  RoPE for all
        # 4 heads is then 6 wide DVE ops instead of 24 narrow ones.
        def strip_epilogue(it, q_ps, kv_ps, cs):
            nc.vector.tensor_add(q_ps, q_ps, bias_sb[:, 0:QW])
            nc.vector.tensor_add(kv_ps, kv_ps, bias_sb[:, QW:QW + KVW])
            # rms stats: per-head two-piece views, one sqrt+recip for all 5
            q3d = q_ps.rearrange("p (two h d) -> p two h d", two=2, h=HP, d=64)
            sq5 = small.tile([128, 8], F32, tag="sq5", name="sq5")
            for b in range(HP + 1):
                src = q3d[:, :, b] if b < HP else kv_ps[:, 0:128]
                sqout = scratch.tile([128, 128], F32, tag="sqout", name="sqout")
                so = (sqout.rearrange("p (two d) -> p two d", two=2)
                      if b < HP else sqout)
                nc.scalar.activation(
                    out=so, in_=src,
                    func=mybir.ActivationFunctionType.Square,
                    accum_out=sq5[:, b:b + 1],
                )
            rstd5 = small.tile([128, 8], F32, tag="rstd5", name="rstd5")
            nc.scalar.activation(
                out=rstd5[:, 0:5], in_=sq5[:, 0:5],
                func=mybir.ActivationFunctionType.Sqrt,
                bias=eps_t, scale=1.0 / D,
            )
            nc.vector.reciprocal_approx_fast(out=rstd5[:, 0:5], in_=rstd5[:, 0:5])
            # q rope, all 4 heads at once: out1 = x1*c1 - x2*s1; out2 = x2*c2 + x1*s2
            x1, x2 = q_ps[:, 0:256], q_ps[:, 256:512]
            qrt = scratch.tile([128, 512], F32, tag="qrt", name="qrt")
            qm = scratch.tile([128, 256], F32, tag="qm", name="qm")
            nc.vector.tensor_mul(qrt[:, 0:256], x1, cs[:, 0:256])
            nc.vector.tensor_mul(qm, x2, cs[:, 512:768])
            nc.vector.tensor_sub(qrt[:, 0:256], qrt[:, 0:256], qm)
            nc.vector.tensor_mul(qrt[:, 256:512], x2, cs[:, 256:512])
            nc.vector.tensor_mul(qm, x1, cs[:, 768:1024])
            nc.vector.tensor_add(qrt[:, 256:512], qrt[:, 256:512], qm)
            qrt3d = qrt.rearrange("p (two h d) -> p two h d", two=2, h=HP, d=64)
            # k rope
            ksrc = kv_ps[:, 0:128]
            c_t, s_t = cs[:, 1024:1152], cs[:, 1152:1280]
            krt = scratch.tile([128, 128], F32, tag="krt", name="krt")
            km = scratch.tile([128, 64], F32, tag="km", name="km")
            nc.vector.tensor_mul(krt[:, 0:64], ksrc[:, 0:64], c_t[:, 0:64])
            nc.vector.tensor_mul(km, ksrc[:, 64:128], s_t[:, 0:64])
            nc.vector.tensor_sub(krt[:, 0:64], krt[:, 0:64], km)
            nc.vector.tensor_mul(krt[:, 64:128], ksrc[:, 64:128], c_t[:, 64:128])
            nc.vector.tensor_mul(km, ksrc[:, 0:64], s_t[:, 64:128])
            nc.vector.tensor_add(krt[:, 64:128], krt[:, 64:128], km)
            for b in range(HP + 1):  # 0..3 q heads, 4 = k
                rb = scratch.tile([128, 128], BF16, tag="rb", name="rb")
                if b < HP:
                    nc.vector.tensor_scalar_mul(
                        rb.rearrange("p (two d) -> p two d", two=2),
                        qrt3d[:, :, b], rstd5[:, b:b + 1])
                else:
                    nc.vector.tensor_scalar_mul(rb, krt, rstd5[:, b:b + 1])
                tp = psC.tile([128, 128], BF16, tag="d", name="tp")
                nc.tensor.transpose(tp, rb, ident)
                dst = qT[b] if b < HP else kT
                nc.scalar.copy(out=dst[:, it * 128:(it + 1) * 128], in_=tp)
            # v
            nc.scalar.copy(out=vsb[it], in_=kv_ps[:, 128:256])

        # strips 0+1 share one e-loop so the PE consumes freshly arriving
        # weight chunks at half rate during the initial weight download
        qkv01 = []
        for it in range(2):
            qkv01.append((psA.tile([128, QW], F32, tag="m", name=f"q_ps{it}"),
                          psB.tile([128, KVW], F32, tag="c", name=f"kv_ps{it}")))
        for e in range(NE):
            for it in range(2):
                xt = xstrips01[it][:, e * 128:(e + 1) * 128]
                q_ps, kv_ps = qkv01[it]
                nc.tensor.matmul(q_ps, xt, wq_sb[:, e * QW:(e + 1) * QW],
                                 start=(e == 0), stop=(e == NE - 1))
                nc.tensor.matmul(kv_ps, xt, wkv_sb[:, e * KVW:(e + 1) * KVW],
                                 start=(e == 0), stop=(e == NE - 1))
        strip_epilogue(0, qkv01[0][0], qkv01[0][1], css01[0])

        # epilogue for strip it-1 is emitted after strip it's matmuls so the
        # PE-side transposes have a full strip of slack behind the DVE chain
        pending = (1, qkv01[1][0], qkv01[1][1], css01[1])
        for it in range(2, NT):
            xstrip = xpool.tile([128, NE * 128], BF16, tag="xstrip",
                                name=f"xstrip{it}")
            r0, r1 = it * 128, (it + 1) * 128
            nc.sync.dma_start(out=xstrip[:, 0:2048], in_=xT_d[r0:r1, 0:2048])
            nc.sync.dma_start(out=xstrip[:, 2048:4096], in_=xT_d[r0:r1, 2048:4096])
            cs = cspool.tile([128, 1280], F32, tag="cs", name=f"cs{it}")
            nc.sync.dma_start(out=cs, in_=cs_d[r0:r1, :])

            q_ps = psA.tile([128, QW], F32, tag="m", name="q_ps")
            kv_ps = psB.tile([128, KVW], F32, tag="c", name="kv_ps")
            for e in range(NE):
                xt = xstrip[:, e * 128:(e + 1) * 128]
                nc.tensor.matmul(q_ps, xt, wq_sb[:, e * QW:(e + 1) * QW],
                                 start=(e == 0), stop=(e == NE - 1))
                nc.tensor.matmul(kv_ps, xt, wkv_sb[:, e * KVW:(e + 1) * KVW],
                                 start=(e == 0), stop=(e == NE - 1))
            strip_epilogue(*pending)
            pending = (it, q_ps, kv_ps, cs)
        strip_epilogue(*pending)

        # ---------------- Phase B + C interleaved per 512-query slice ------
        LOOKAHEAD = 3
        for si in range(T // 512):
            njb = 4 * si + 4
            blocks = [(h, jb) for h in range(HP) for jb in range(njb)]
            s_tiles = {}

            def emit_s(idx):
                h, jb = blocks[idx]
                kk = jb - 4 * si
                # masked-out query columns of diagonal blocks are skipped
                q0 = 128 * kk if kk > 0 else 0
                s_ps = psA.tile([128, 512], F32, tag="m", name="s_ps")
                nc.tensor.matmul(
                    s_ps[:, q0:512], kT[:, jb * 128:(jb + 1) * 128],
                    qT[h][:, si * 512 + q0:(si + 1) * 512],
                    start=True, stop=True,
                )
                if kk >= 0:  # diagonal (partially masked) block
                    nc.vector.tensor_add(
                        s_ps[:, q0:512], s_ps[:, q0:512],
                        mask_sb[:, 384:384 + 512 - q0])
                s_tiles[idx] = (s_ps, q0)

            for idx in range(min(LOOKAHEAD, len(blocks))):
                emit_s(idx)
            ctx_ps = pacc = None
            for i, (h, jb) in enumerate(blocks):
                s_ps, q0 = s_tiles.pop(i)
                p_t = ppool.tile([128, 512], BF16, tag="pt", name="pt")
                nc.scalar.activation(
                    out=p_t[:, q0:512], in_=s_ps[:, q0:512],
                    func=mybir.ActivationFunctionType.Exp,
                    scale=SM_SCALE,
                )
                if i + LOOKAHEAD < len(blocks):
                    emit_s(i + LOOKAHEAD)
                if jb == 0:
                    ctx_ps = psB.tile([128, 512], F32, tag="c", name="ctx_ps")
                    pacc = epool.tile([128, 512], F32, tag="pacc", name="pacc")
                    nc.vector.tensor_copy(out=pacc, in_=p_t)
                else:
                    nc.vector.tensor_add(pacc[:, q0:512], pacc[:, q0:512],
                                         p_t[:, q0:512])
                nc.tensor.matmul(ctx_ps[:, q0:512], vsb[jb], p_t[:, q0:512],
                                 start=(jb == 0), stop=(jb == njb - 1),
                                 skip_group_check=True)
                if jb == njb - 1:
                    # denominator: one f32r matmul broadcasts the partition
                    # sum of the accumulated exp mass to all 128 partitions
                    den_ps = psC.tile([128, 512], F32, tag="d", name="den_ps")
                    nc.tensor.matmul(den_ps, ones_f32, pacc,
                                     start=True, stop=True)
                    rden = epool.tile([128, 512], F32, tag="rden", name="rden")
                    nc.vector.reciprocal_approx_fast(out=rden, in_=den_ps)
                    nc.vector.tensor_mul(
                        ctxT[h][:, si * 512:(si + 1) * 512], ctx_ps, rden)

            # out projection for this slice's 4 row blocks
            for it in range(4 * si, 4 * si + 4):
                osb = opool.tile([128, EMB], BF16, tag="osb", name="osb")
                for ot in range(NO):
                    o_ps = psA.tile([128, 512], F32, tag="m", name="o_ps")
                    for hh in range(HP):
                        nc.tensor.matmul(
                            o_ps,
                            ctxT[hh][:, it * 128:(it + 1) * 128],
                            wo_sb[:, hh * EMB + ot * 512:hh * EMB + (ot + 1) * 512],
                            start=(hh == 0), stop=(hh == HP - 1),
                        )
                    nc.scalar.copy(out=osb[:, ot * 512:(ot + 1) * 512], in_=o_ps)
                nc.sync.dma_start(
                    out=out_d[it * 128:(it + 1) * 128, :], in_=osb)

    return nc


def _prep_inputs(x, mask, cos, sin, wq, bq, wk, bk, wv, bv, wo, q_scale, k_scale):
    x2 = np.asarray(x, dtype=np.float32).reshape(T, EMB)
    # strip layout: row (it*128 + p), col (eb*128 + t) holds x[it*128+t, eb*128+p]
    xTt = x2.reshape(NT, 128, NE, 128).transpose(0, 3, 2, 1)
    xTt = np.ascontiguousarray(xTt).reshape(NT * 128, NE * 128).astype(BF)

    qs = np.asarray(q_scale, dtype=np.float32)
    ks = np.asarray(k_scale, dtype=np.float32)
    qs_rot = np.concatenate([qs[64:], qs[:64]])
    ks_rot = np.concatenate([ks[64:], ks[:64]])
    cos = np.asarray(cos, dtype=np.float32)
    sin = np.asarray(sin, dtype=np.float32)
    cosq = cos * qs[None, :]
    sinq = sin * qs_rot[None, :]
    # q tables tiled 4-wide: [c1 x4 | c2 x4], matching half-split q layout
    cq4 = np.concatenate([np.tile(cosq[:, 0:64], (1, HP)),
                          np.tile(cosq[:, 64:128], (1, HP))], axis=1)
    sq4 = np.concatenate([np.tile(sinq[:, 0:64], (1, HP)),
                          np.tile(sinq[:, 64:128], (1, HP))], axis=1)
    cs = np.concatenate([cq4, sq4, cos * ks[None, :], sin * ks_rot[None, :]],
                        axis=1)
    cs = np.ascontiguousarray(cs, dtype=np.float32)

    def q_halfsplit(a):
        # permute last axis from [h][half][d'] to [half][h][d']
        return (a.reshape(*a.shape[:-1], HP, 2, 64)
                .swapaxes(-3, -2)
                .reshape(*a.shape))

    jj = np.arange(128)[:, None]
    cc = np.arange(896)[None, :]
    maskT = np.where(jj > cc - 384, NEG, 0.0).astype(np.float32)

    wq = np.asarray(wq, dtype=np.float32)
    wk = np.asarray(wk, dtype=np.float32)
    wv = np.asarray(wv, dtype=np.float32)
    wo = np.asarray(wo, dtype=np.float32)
    bq = np.asarray(bq, dtype=np.float32)
    bk = np.asarray(bk, dtype=np.float32)
    bv = np.asarray(bv, dtype=np.float32)

    in_maps = []
    for c in range(NCORES):
        # [p, e*QW + o] = wq[e*128 + p, c*QW + perm(o)]
        wq_c = q_halfsplit(wq[:, c * QW:(c + 1) * QW]).reshape(NE, 128, QW)
        wq_c = np.ascontiguousarray(wq_c.transpose(1, 0, 2)).reshape(128, NE * QW)
        wkv_c = np.concatenate(
            [wk[:, c * D:(c + 1) * D], wv[:, c * D:(c + 1) * D]], axis=1)
        wkv_c = wkv_c.reshape(NE, 128, KVW)
        wkv_c = np.ascontiguousarray(wkv_c.transpose(1, 0, 2)).reshape(128, NE * KVW)
        # [p, h*EMB + col] = wo[c*QW + h*128 + p, col]
        wo_c = wo[c * QW:(c + 1) * QW, :].reshape(HP, 128, EMB)
        wo_c = np.ascontiguousarray(wo_c.transpose(1, 0, 2)).reshape(128, HP * EMB)
        bias_c = np.broadcast_to(
            np.concatenate([q_halfsplit(bq[c * QW:(c + 1) * QW]),
                            bk[c * D:(c + 1) * D], bv[c * D:(c + 1) * D]]),
            (128, QW + KVW))
        in_maps.append({
            "xT": xTt,
            "wq": wq_c.astype(BF),
            "wkv": wkv_c.astype(BF),
            "wo": wo_c.astype(BF),
            "cs": cs,
            "maskT": maskT,
            "biasb": np.ascontiguousarray(bias_c, dtype=np.float32),
        })
    return in_maps


def _get_program():
    if "nc" not in _prog_cache:
        nc = _build_program()
        if not nc.is_finalized():
            nc.finalize()
        _prog_cache["nc"] = nc
    return _prog_cache["nc"]


def kernel(**inputs):
    in_maps = _prep_inputs(**inputs)
    nc = _get_program()
    res = run_bass_kernel_spmd(nc, in_maps, list(range(NCORES)))
    out = np.zeros((T, EMB), dtype=np.float32)
    for r in res.results:
        out += np.asarray(r["out"], dtype=np.float32)
    return out.reshape(1, T, EMB)


# revision 36
# speedup vs baseline: 1.3366x; 1.3366x over previous
"""GQA attention kernel for Trainium2, tensor-parallel over heads on 8 cores.

Problem: B=1, T=2048, EMB=4096, H=32 query heads, G=8 KV groups, D=128.
Reference: q/k/v projections -> per-head RMS norm (q,k) -> RoPE (q,k) ->
causal GQA attention -> out projection.

Sharding: core c owns query heads [4c, 4c+4) and KV group c.  Each core
computes a partial output for its heads; host sums the 8 partials (the
all-reduce of the module's TP scheme, done on host since full I/O is
required anyway).

Optimization history (709us baseline -> ~427us measured):
  - Phase B runs as one flat software pipeline per 512-query slice across
    all 4 heads: S matmuls emitted 3 blocks ahead of the exp->ctx chain so
    the PE never stalls on ACT latency and holds its max p-state clock.
  - Out-projection units of slice si-1 are interleaved into B(si)'s block
    pipeline, filling the PE during B's exp(ACT)-bound stretches and
    draining output DMA throughout attention.
  - Diagonal (causal) blocks narrowed to their unmasked query columns in
    S/mask/exp/ctx; softmax denominator = DVE-accumulated exp mass + one
    broadcast matmul (all-ones stationary) per group, off the PE hot path.
  - reciprocal_approx_fast (5x faster DVE reciprocal, 18-bit) for both
    softmax denominators and rms rstd (batched: one sqrt+recip per strip).
  - q stored half-split head-interleaved (all first-halves then second
    halves) so RoPE for 4 heads is 6 wide DVE ops instead of 24 narrow.
  - Each strip's rms/rope/transpose epilogue is emitted one strip late so
    the PE-side transposes have a strip of slack behind the DVE chain.
  - All weights pre-swizzled host-side to partition-major layouts, posted
    as ~512KB chunks in e-order; strips 0+1 share one e-loop so the PE
    consumes freshly arriving weight chunks at half rate during startup.
  - bf16 partial outputs (halves out-DMA; host sums partials in fp32).
"""

import numpy as np
import ml_dtypes
from contextlib import ExitStack

import concourse.bass as bass
import concourse.bacc as bacc
import concourse.mybir as mybir
from concourse.tile import TileContext
from concourse.bass_utils import run_bass_kernel_spmd
from concourse.masks import make_identity

EMB, H, G, D, T = 4096, 32, 8, 128, 2048
EPS = 1e-6
NCORES = 8
HP = H // NCORES          # 4 query heads per core
NT = T // 128             # 16 t-tiles
NE = EMB // 128           # 32 e-tiles
NO = EMB // 512           # 8 output column tiles
QW = HP * D               # 512 = q width per core
KVW = 2 * D               # 256 = k|v width per core
SM_SCALE = 1.0 / float(np.sqrt(D))
NEG = -1e9

F32 = mybir.dt.float32
BF16 = mybir.dt.bfloat16
BF = ml_dtypes.bfloat16

_prog_cache = {}


def _build_program():
    nc = bacc.Bacc()

    xT_d = nc.declare_dram_parameter("xT", [NT * 128, NE * 128], BF16, isOutput=False)
    wq_d = nc.declare_dram_parameter("wq", [128, NE * QW], BF16, isOutput=False)
    wkv_d = nc.declare_dram_parameter("wkv", [128, NE * KVW], BF16, isOutput=False)
    wo_d = nc.declare_dram_parameter("wo", [128, HP * EMB], BF16, isOutput=False)
    # [cosq x4 | sinq x4 | cosk | sink]; q tables tiled 4-wide to match the
    # half-split head-interleaved q layout (all first-halves, then second)
    cs_d = nc.declare_dram_parameter("cs", [NT * 128, 1280], F32, isOutput=False)
    mask_d = nc.declare_dram_parameter("maskT", [128, 896], F32, isOutput=False)
    bias_d = nc.declare_dram_parameter("biasb", [128, QW + KVW], F32, isOutput=False)
    out_d = nc.declare_dram_parameter("out", [T, EMB], BF16, isOutput=True)

    with TileContext(nc) as tc, ExitStack() as ctx:
        consts = ctx.enter_context(tc.tile_pool(name="consts", bufs=1))
        wpool = ctx.enter_context(tc.tile_pool(name="wpool", bufs=1))
        xpool = ctx.enter_context(tc.tile_pool(name="xpool", bufs=3))
        cspool = ctx.enter_context(tc.tile_pool(name="cspool", bufs=2))
        scratch = ctx.enter_context(tc.tile_pool(name="scratch", bufs=3))
        small = ctx.enter_context(tc.tile_pool(name="small", bufs=4))
        ppool = ctx.enter_context(tc.tile_pool(name="ppool", bufs=4))
        epool = ctx.enter_context(tc.tile_pool(name="epool", bufs=2))
        opool = ctx.enter_context(tc.tile_pool(name="opool", bufs=2))
        resid = ctx.enter_context(tc.tile_pool(name="resid", bufs=1))
        psA = ctx.enter_context(tc.tile_pool(name="psA", bufs=4, space="PSUM"))
        psB = ctx.enter_context(tc.tile_pool(name="psB", bufs=2, space="PSUM"))
        psC = ctx.enter_context(tc.tile_pool(name="psC", bufs=2, space="PSUM"))

        # engine-side constants (no DMA involved)
        ident = consts.tile([128, 128], BF16, tag="ident", name="ident")
        make_identity(nc, ident)
        ones_f32 = consts.tile([128, 128], F32, tag="ones_f32", name="ones_f32")
        nc.vector.memset(ones_f32, 1.0)
        eps_t = consts.tile([128, 1], F32, tag="eps", name="eps")
        nc.vector.memset(eps_t, EPS)

        # strips 0/1 inputs first so phase A can start while weights stream in
        xstrips01 = []
        css01 = []
        for it in range(2):
            xs = xpool.tile([128, NE * 128], BF16, tag="xstrip", name=f"xstrip{it}")
            r0, r1 = it * 128, (it + 1) * 128
            nc.sync.dma_start(out=xs[:, 0:2048], in_=xT_d[r0:r1, 0:2048])
            nc.sync.dma_start(out=xs[:, 2048:4096], in_=xT_d[r0:r1, 2048:4096])
            cst = cspool.tile([128, 1280], F32, tag="cs", name=f"cs{it}")
            nc.sync.dma_start(out=cst, in_=cs_d[r0:r1, :])
            xstrips01.append(xs)
            css01.append(cst)
        bias_sb = consts.tile([128, QW + KVW], F32, tag="bias", name="bias")
        nc.sync.dma_start(out=bias_sb, in_=bias_d[:, :])

        # resident weights, posted in e-order in ~512KB chunks
        wq_sb = wpool.tile([128, NE * QW], BF16, tag="wq", name="wq")
        wkv_sb = wpool.tile([128, NE * KVW], BF16, tag="wkv", name="wkv")
        for ch in range(8):
            c0, c1 = ch * 4 * QW, (ch + 1) * 4 * QW
            nc.sync.dma_start(out=wq_sb[:, c0:c1], in_=wq_d[:, c0:c1])
            if ch % 2 == 0:
                k0, k1 = ch * 4 * KVW, (ch + 2) * 4 * KVW
                nc.sync.dma_start(out=wkv_sb[:, k0:k1], in_=wkv_d[:, k0:k1])
        mask_sb = consts.tile([128, 896], F32, tag="mask", name="mask")
        nc.sync.dma_start(out=mask_sb, in_=mask_d[:, :])

        # out-proj weights last (not needed until phase C)
        wo_sb = wpool.tile([128, HP * EMB], BF16, tag="wo", name="wo")
        nc.sync.dma_start(out=wo_sb[:, 0:HP * EMB // 2], in_=wo_d[:, 0:HP * EMB // 2])
        nc.sync.dma_start(out=wo_sb[:, HP * EMB // 2:], in_=wo_d[:, HP * EMB // 2:])

        # resident activations
        qT = [resid.tile([128, T], BF16, tag=f"qT{h}", name=f"qT{h}") for h in range(HP)]
        kT = resid.tile([128, T], BF16, tag="kT", name="kT")
        vsb = [resid.tile([128, 128], BF16, tag=f"v{j}", name=f"v{j}") for j in range(NT)]
        ctxT = [resid.tile([128, T], BF16, tag=f"ctxT{h}", name=f"ctxT{h}") for h in range(HP)]

        # ---------------- Phase A: projections + rms + rope + transpose ----
        # q layout is half-split head-interleaved: cols [h*64+d'] hold head
        # h dims 0..63, cols [256+h*64+d'] hold dims 64..127.  RoPE for all
        # 4 heads is then 6 wide DVE ops instead of 24 narrow ones.
        def strip_epilogue(it, q_ps, kv_ps, cs):
            nc.vector.tensor_add(q_ps, q_ps, bias_sb[:, 0:QW])
            nc.vector.tensor_add(kv_ps, kv_ps, bias_sb[:, QW:QW + KVW])
            # rms stats: per-head two-piece views, one sqrt+recip for all 5
            q3d = q_ps.rearrange("p (two h d) -> p two h d", two=2, h=HP, d=64)
            sq5 = small.tile([128, 8], F32, tag="sq5", name="sq5")
            for b in range(HP + 1):
                src = q3d[:, :, b] if b < HP else kv_ps[:, 0:128]
                sqout = scratch.tile([128, 128], F32, tag="sqout", name="sqout")
                so = (sqout.rearrange("p (two d) -> p two d", two=2)
                      if b < HP else sqout)
                nc.scalar.activation(
                    out=so, in_=src,
                    func=mybir.ActivationFunctionType.Square,
                    accum_out=sq5[:, b:b + 1],
                )
            rstd5 = small.tile([128, 8], F32, tag="rstd5", name="rstd5")
            nc.scalar.activation(
                out=rstd5[:, 0:5], in_=sq5[:, 0:5],
                func=mybir.ActivationFunctionType.Sqrt,
                bias=eps_t, scale=1.0 / D,
            )
            nc.vector.reciprocal_approx_fast(out=rstd5[:, 0:5], in_=rstd5[:, 0:5])
            # q rope, all 4 heads at once: out1 = x1*c1 - x2*s1; out2 = x2*c2 + x1*s2
            x1, x2 = q_ps[:, 0:256], q_ps[:, 256:512]
            qrt = scratch.tile([128, 512], F32, tag="qrt", name="qrt")
            qm = scratch.tile([128, 256], F32, tag="qm", name="qm")
            nc.vector.tensor_mul(qrt[:, 0:256], x1, cs[:, 0:256])
            nc.vector.tensor_mul(qm, x2, cs[:, 512:768])
            nc.vector.tensor_sub(qrt[:, 0:256], qrt[:, 0:256], qm)
            nc.vector.tensor_mul(qrt[:, 256:512], x2, cs[:, 256:512])
            nc.vector.tensor_mul(qm, x1, cs[:, 768:1024])
            nc.vector.tensor_add(qrt[:, 256:512], qrt[:, 256:512], qm)
            qrt3d = qrt.rearrange("p (two h d) -> p two h d", two=2, h=HP, d=64)
            # k rope
            ksrc = kv_ps[:, 0:128]
            c_t, s_t = cs[:, 1024:1152], cs[:, 1152:1280]
            krt = scratch.tile([128, 128], F32, tag="krt", name="krt")
            km = scratch.tile([128, 64], F32, tag="km", name="km")
            nc.vector.tensor_mul(krt[:, 0:64], ksrc[:, 0:64], c_t[:, 0:64])
            nc.vector.tensor_mul(km, ksrc[:, 64:128], s_t[:, 0:64])
            nc.vector.tensor_sub(krt[:, 0:64], krt[:, 0:64], km)
            nc.vector.tensor_mul(krt[:, 64:128], ksrc[:, 64:128], c_t[:, 64:128])
            nc.vector.tensor_mul(km, ksrc[:, 0:64], s_t[:, 64:128])
            nc.vector.tensor_add(krt[:, 64:128], krt[:, 64:128], km)
            for b in range(HP + 1):  # 0..3 q heads, 4 = k
                rb = scratch.tile([128, 128], BF16, tag="rb", name="rb")
                if b < HP:
                    nc.vector.tensor_scalar_mul(
                        rb.rearrange("p (two d) -> p two d", two=2),
                        qrt3d[:, :, b], rstd5[:, b:b + 1])
                else:
                    nc.vector.tensor_scalar_mul(rb, krt, rstd5[:, b:b + 1])
                tp = psC.tile([128, 128], BF16, tag="d", name="tp")
                nc.tensor.transpose(tp, rb, ident)
                dst = qT[b] if b < HP else kT
                nc.scalar.copy(out=dst[:, it * 128:(it + 1) * 128], in_=tp)
            # v
            nc.scalar.copy(out=vsb[it], in_=kv_ps[:, 128:256])

        # strips 0+1 share one e-loop so the PE consumes freshly arriving
        # weight chunks at half rate during the initial weight download
        qkv01 = []
        for it in range(2):
            qkv01.append((psA.tile([128, QW], F32, tag="m", name=f"q_ps{it}"),
                          psB.tile([128, KVW], F32, tag="c", name=f"kv_ps{it}")))
        for e in range(NE):
            for it in range(2):
                xt = xstrips01[it][:, e * 128:(e + 1) * 128]
                q_ps, kv_ps = qkv01[it]
                nc.tensor.matmul(q_ps, xt, wq_sb[:, e * QW:(e + 1) * QW],
                                 start=(e == 0), stop=(e == NE - 1))
                nc.tensor.matmul(kv_ps, xt, wkv_sb[:, e * KVW:(e + 1) * KVW],
                                 start=(e == 0), stop=(e == NE - 1))
        strip_epilogue(0, qkv01[0][0], qkv01[0][1], css01[0])

        # epilogue for strip it-1 is emitted after strip it's matmuls so the
        # PE-side transposes have a full strip of slack behind the DVE chain
        pending = (1, qkv01[1][0], qkv01[1][1], css01[1])
        for it in range(2, NT):
            xstrip = xpool.tile([128, NE * 128], BF16, tag="xstrip",
                                name=f"xstrip{it}")
            r0, r1 = it * 128, (it + 1) * 128
            nc.sync.dma_start(out=xstrip[:, 0:2048], in_=xT_d[r0:r1, 0:2048])
            nc.sync.dma_start(out=xstrip[:, 2048:4096], in_=xT_d[r0:r1, 2048:4096])
            cs = cspool.tile([128, 1280], F32, tag="cs", name=f"cs{it}")
            nc.sync.dma_start(out=cs, in_=cs_d[r0:r1, :])

            q_ps = psA.tile([128, QW], F32, tag="m", name="q_ps")
            kv_ps = psB.tile([128, KVW], F32, tag="c", name="kv_ps")
            for e in range(NE):
                xt = xstrip[:, e * 128:(e + 1) * 128]
                nc.tensor.matmul(q_ps, xt, wq_sb[:, e * QW:(e + 1) * QW],
                                 start=(e == 0), stop=(e == NE - 1))
                nc.tensor.matmul(kv_ps, xt, wkv_sb[:, e * KVW:(e + 1) * KVW],
                                 start=(e == 0), stop=(e == NE - 1))
            strip_epilogue(*pending)
            pending = (it, q_ps, kv_ps, cs)
        strip_epilogue(*pending)

        # ---------------- Phase B + C interleaved per 512-query slice ------
        # C units (one (it, ot) out-proj group each) of slice si-1 are
        # interleaved into B(si)'s block pipeline so the PE stays fed
        # through B's ACT(exp)-bound stretches.
        LOOKAHEAD = 3

        def c_unit(it, ot, osb):
            o_ps = psA.tile([128, 512], F32, tag="m", name="o_ps")
            for hh in range(HP):
                nc.tensor.matmul(
                    o_ps,
                    ctxT[hh][:, it * 128:(it + 1) * 128],
                    wo_sb[:, hh * EMB + ot * 512:hh * EMB + (ot + 1) * 512],
                    start=(hh == 0), stop=(hh == HP - 1),
                )
            nc.scalar.copy(out=osb[:, ot * 512:(ot + 1) * 512], in_=o_ps)
            if ot == NO - 1:
                nc.sync.dma_start(
                    out=out_d[it * 128:(it + 1) * 128, :], in_=osb)

        def c_units_for(si):
            units = []
            for it in range(4 * si, 4 * si + 4):
                osb = opool.tile([128, EMB], BF16, tag="osb", name="osb")
                for ot in range(NO):
                    units.append((it, ot, osb))
            return units

        for si in range(T // 512):
            njb = 4 * si + 4
            blocks = [(h, jb) for h in range(HP) for jb in range(njb)]
            pending_c = c_units_for(si - 1) if si > 0 else []
            pace = max(1, (len(blocks) + len(pending_c) - 1) //
                       max(1, len(pending_c))) if pending_c else 0
            s_tiles = {}

            def emit_s(idx):
                h, jb = blocks[idx]
                kk = jb - 4 * si
                # masked-out query columns of diagonal blocks are skipped
                q0 = 128 * kk if kk > 0 else 0
                s_ps = psA.tile([128, 512], F32, tag="m", name="s_ps")
                nc.tensor.matmul(
                    s_ps[:, q0:512], kT[:, jb * 128:(jb + 1) * 128],
                    qT[h][:, si * 512 + q0:(si + 1) * 512],
                    start=True, stop=True,
                )
                if kk >= 0:  # diagonal (partially masked) block
                    nc.vector.tensor_add(
                        s_ps[:, q0:512], s_ps[:, q0:512],
                        mask_sb[:, 384:384 + 512 - q0])
                s_tiles[idx] = (s_ps, q0)

            for idx in range(min(LOOKAHEAD, len(blocks))):
                emit_s(idx)
            ctx_ps = pacc = None
            for i, (h, jb) in enumerate(blocks):
                s_ps, q0 = s_tiles.pop(i)
                p_t = ppool.tile([128, 512], BF16, tag="pt", name="pt")
                nc.scalar.activation(
                    out=p_t[:, q0:512], in_=s_ps[:, q0:512],
                    func=mybir.ActivationFunctionType.Exp,
                    scale=SM_SCALE,
                )
                if i + LOOKAHEAD < len(blocks):
                    emit_s(i + LOOKAHEAD)
                if jb == 0:
                    ctx_ps = psB.tile([128, 512], F32, tag="c", name="ctx_ps")
                    pacc = epool.tile([128, 512], F32, tag="pacc", name="pacc")
                    nc.vector.tensor_copy(out=pacc, in_=p_t)
                else:
                    nc.vector.tensor_add(pacc[:, q0:512], pacc[:, q0:512],
                                         p_t[:, q0:512])
                nc.tensor.matmul(ctx_ps[:, q0:512], vsb[jb], p_t[:, q0:512],
                                 start=(jb == 0), stop=(jb == njb - 1),
                                 skip_group_check=True)
                if pending_c and pace and i % pace == pace - 1:
                    c_unit(*pending_c.pop(0))
                if jb == njb - 1:
                    # denominator: one f32r matmul broadcasts the partition
                    # sum of the accumulated exp mass to all 128 partitions
                    den_ps = psC.tile([128, 512], F32, tag="d", name="den_ps")
                    nc.tensor.matmul(den_ps, ones_f32, pacc,
                                     start=True, stop=True)
                    rden = epool.tile([128, 512], F32, tag="rden", name="rden")
                    nc.vector.reciprocal_approx_fast(out=rden, in_=den_ps)
                    nc.vector.tensor_mul(
                        ctxT[h][:, si * 512:(si + 1) * 512], ctx_ps, rden)

            # leftover C units of si-1 not consumed by the pacing
            for u in pending_c:
                c_unit(*u)

        # final slice's out projection (no following B to interleave with)
        for u in c_units_for(T // 512 - 1):
            c_unit(*u)

    return nc


def _prep_inputs(x, mask, cos, sin, wq, bq, wk, bk, wv, bv, wo, q_scale, k_scale):
    x2 = np.asarray(x, dtype=np.float32).reshape(T, EMB)
    # strip layout: row (it*128 + p), col (eb*128 + t) holds x[it*128+t, eb*128+p]
    xTt = x2.reshape(NT, 128, NE, 128).transpose(0, 3, 2, 1)
    xTt = np.ascontiguousarray(xTt).reshape(NT * 128, NE * 128).astype(BF)

    qs = np.asarray(q_scale, dtype=np.float32)
    ks = np.asarray(k_scale, dtype=np.float32)
    qs_rot = np.concatenate([qs[64:], qs[:64]])
    ks_rot = np.concatenate([ks[64:], ks[:64]])
    cos = np.asarray(cos, dtype=np.float32)
    sin = np.asarray(sin, dtype=np.float32)
    cosq = cos * qs[None, :]
    sinq = sin * qs_rot[None, :]
    # q tables tiled 4-wide: [c1 x4 | c2 x4], matching half-split q layout
    cq4 = np.concatenate([np.tile(cosq[:, 0:64], (1, HP)),
                          np.tile(cosq[:, 64:128], (1, HP))], axis=1)
    sq4 = np.concatenate([np.tile(sinq[:, 0:64], (1, HP)),
                          np.tile(sinq[:, 64:128], (1, HP))], axis=1)
    cs = np.concatenate([cq4, sq4, cos * ks[None, :], sin * ks_rot[None, :]],
                        axis=1)
    cs = np.ascontiguousarray(cs, dtype=np.float32)

    def q_halfsplit(a):
        # permute last axis from [h][half][d'] to [half][h][d']
        return (a.reshape(*a.shape[:-1], HP, 2, 64)
                .swapaxes(-3, -2)
                .reshape(*a.shape))

    jj = np.arange(128)[:, None]
    cc = np.arange(896)[None, :]
    maskT = np.where(jj > cc - 384, NEG, 0.0).astype(np.float32)

    wq = np.asarray(wq, dtype=np.float32)
    wk = np.asarray(wk, dtype=np.float32)
    wv = np.asarray(wv, dtype=np.float32)
    wo = np.asarray(wo, dtype=np.float32)
    bq = np.asarray(bq, dtype=np.float32)
    bk = np.asarray(bk, dtype=np.float32)
    bv = np.asarray(bv, dtype=np.float32)

    in_maps = []
    for c in range(NCORES):
        # [p, e*QW + o] = wq[e*128 + p, c*QW + perm(o)]
        wq_c = q_halfsplit(wq[:, c * QW:(c + 1) * QW]).reshape(NE, 128, QW)
        wq_c = np.ascontiguousarray(wq_c.transpose(1, 0, 2)).reshape(128, NE * QW)
        wkv_c = np.concatenate(
            [wk[:, c * D:(c + 1) * D], wv[:, c * D:(c + 1) * D]], axis=1)
        wkv_c = wkv_c.reshape(NE, 128, KVW)
        wkv_c = np.ascontiguousarray(wkv_c.transpose(1, 0, 2)).reshape(128, NE * KVW)
        # [p, h*EMB + col] = wo[c*QW + h*128 + p, col]
        wo_c = wo[c * QW:(c + 1) * QW, :].reshape(HP, 128, EMB)
        wo_c = np.ascontiguousarray(wo_c.transpose(1, 0, 2)).reshape(128, HP * EMB)
        bias_c = np.broadcast_to(
            np.concatenate([q_halfsplit(bq[c * QW:(c + 1) * QW]),
                            bk[c * D:(c + 1) * D], bv[c * D:(c + 1) * D]]),
            (128, QW + KVW))
        in_maps.append({
            "xT": xTt,
            "wq": wq_c.astype(BF),
            "wkv": wkv_c.astype(BF),
            "wo": wo_c.astype(BF),
            "cs": cs,
            "maskT": maskT,
            "biasb": np.ascontiguousarray(bias_c, dtype=np.float32),
        })
    return in_maps


def _get_program():
    if "nc" not in _prog_cache:
        nc = _build_program()
        if not nc.is_finalized():
            nc.finalize()
        _prog_cache["nc"] = nc
    return _prog_cache["nc"]


def kernel(**inputs):
    in_maps = _prep_inputs(**inputs)
    nc = _get_program()
    res = run_bass_kernel_spmd(nc, in_maps, list(range(NCORES)))
    out = np.zeros((T, EMB), dtype=np.float32)
    for r in res.results:
        out += np.asarray(r["out"], dtype=np.float32)
    return out.reshape(1, T, EMB)


# revision 42
# speedup vs baseline: 1.3423x; 1.0042x over previous
"""GQA attention kernel for Trainium2, tensor-parallel over heads on 8 cores.

Problem: B=1, T=2048, EMB=4096, H=32 query heads, G=8 KV groups, D=128.
Reference: q/k/v projections -> per-head RMS norm (q,k) -> RoPE (q,k) ->
causal GQA attention -> out projection.

Sharding: core c owns query heads [4c, 4c+4) and KV group c.  Each core
computes a partial output for its heads; host sums the 8 partials (the
all-reduce of the module's TP scheme, done on host since full I/O is
required anyway).

Optimization history (709us baseline -> ~427us measured):
  - Phase B runs as one flat software pipeline per 512-query slice across
    all 4 heads: S matmuls emitted 3 blocks ahead of the exp->ctx chain so
    the PE never stalls on ACT latency and holds its max p-state clock.
  - Out-projection units of slice si-1 are interleaved into B(si)'s block
    pipeline, filling the PE during B's exp(ACT)-bound stretches and
    draining output DMA throughout attention.
  - Diagonal (causal) blocks narrowed to their unmasked query columns in
    S/mask/exp/ctx; softmax denominator = DVE-accumulated exp mass + one
    broadcast matmul (all-ones stationary) per group, off the PE hot path.
  - reciprocal_approx_fast (5x faster DVE reciprocal, 18-bit) for both
    softmax denominators and rms rstd (batched: one sqrt+recip per strip).
  - q stored half-split head-interleaved (all first-halves then second
    halves) so RoPE for 4 heads is 6 wide DVE ops instead of 24 narrow.
  - Each strip's rms/rope/transpose epilogue is emitted one strip late so
    the PE-side transposes have a strip of slack behind the DVE chain.
  - All weights pre-swizzled host-side to partition-major layouts, posted
    as ~512KB chunks in e-order; strips 0+1 share one e-loop so the PE
    consumes freshly arriving weight chunks at half rate during startup.
  - bf16 partial outputs (halves out-DMA; host sums partials in fp32).
"""

import numpy as np
import ml_dtypes
from contextlib import ExitStack

import concourse.bass as bass
import concourse.bacc as bacc
import concourse.mybir as mybir
from concourse.tile import TileContext
from concourse.bass_utils import run_bass_kernel_spmd
from concourse.masks import make_identity

EMB, H, G, D, T = 4096, 32, 8, 128, 2048
EPS = 1e-6
NCORES = 8
HP = H // NCORES          # 4 query heads per core
NT = T // 128             # 16 t-tiles
NE = EMB // 128           # 32 e-tiles
NO = EMB // 512           # 8 output column tiles
QW = HP * D               # 512 = q width per core
KVW = 2 * D               # 256 = k|v width per core
SM_SCALE = 1.0 / float(np.sqrt(D))
NEG = -1e9

F32 = mybir.dt.float32
BF16 = mybir.dt.bfloat16
BF = ml_dtypes.bfloat16

_prog_cache = {}


def _build_program():
    nc = bacc.Bacc()

    xT_d = nc.declare_dram_parameter("xT", [NT * 128, NE * 128], BF16, isOutput=False)
    wq_d = nc.declare_dram_parameter("wq", [128, NE * QW], BF16, isOutput=False)
    wkv_d = nc.declare_dram_parameter("wkv", [128, NE * KVW], BF16, isOutput=False)
    wo_d = nc.declare_dram_parameter("wo", [128, HP * EMB], BF16, isOutput=False)
    # [cosq x4 | sinq x4 | cosk | sink]; q tables tiled 4-wide to match the
    # half-split head-interleaved q layout (all first-halves, then second)
    cs_d = nc.declare_dram_parameter("cs", [NT * 128, 1280], F32, isOutput=False)
    mask_d = nc.declare_dram_parameter("maskT", [128, 896], F32, isOutput=False)
    bias_d = nc.declare_dram_parameter("biasb", [128, QW + KVW], F32, isOutput=False)
    out_d = nc.declare_dram_parameter("out", [T, EMB], BF16, isOutput=True)

    with TileContext(nc) as tc, ExitStack() as ctx:
        consts = ctx.enter_context(tc.tile_pool(name="consts", bufs=1))
        wpool = ctx.enter_context(tc.tile_pool(name="wpool", bufs=1))
        xpool = ctx.enter_context(tc.tile_pool(name="xpool", bufs=3))
        cspool = ctx.enter_context(tc.tile_pool(name="cspool", bufs=2))
        scratch = ctx.enter_context(tc.tile_pool(name="scratch", bufs=3))
        small = ctx.enter_context(tc.tile_pool(name="small", bufs=4))
        ppool = ctx.enter_context(tc.tile_pool(name="ppool", bufs=4))
        epool = ctx.enter_context(tc.tile_pool(name="epool", bufs=2))
        opool = ctx.enter_context(tc.tile_pool(name="opool", bufs=2))
        resid = ctx.enter_context(tc.tile_pool(name="resid", bufs=1))
        psA = ctx.enter_context(tc.tile_pool(name="psA", bufs=4, space="PSUM"))
        psB = ctx.enter_context(tc.tile_pool(name="psB", bufs=2, space="PSUM"))
        psC = ctx.enter_context(tc.tile_pool(name="psC", bufs=2, space="PSUM"))

        # engine-side constants (no DMA involved)
        ident = consts.tile([128, 128], BF16, tag="ident", name="ident")
        make_identity(nc, ident)
        ones_f32 = consts.tile([128, 128], F32, tag="ones_f32", name="ones_f32")
        nc.vector.memset(ones_f32, 1.0)
        eps_t = consts.tile([128, 1], F32, tag="eps", name="eps")
        nc.vector.memset(eps_t, EPS)

        # strips 0/1 inputs first so phase A can start while weights stream in
        xstrips01 = []
        css01 = []
        for it in range(2):
            xs = xpool.tile([128, NE * 128], BF16, tag="xstrip", name=f"xstrip{it}")
            r0, r1 = it * 128, (it + 1) * 128
            nc.sync.dma_start(out=xs[:, 0:2048], in_=xT_d[r0:r1, 0:2048])
            nc.sync.dma_start(out=xs[:, 2048:4096], in_=xT_d[r0:r1, 2048:4096])
            cst = cspool.tile([128, 1280], F32, tag="cs", name=f"cs{it}")
            nc.sync.dma_start(out=cst, in_=cs_d[r0:r1, :])
            xstrips01.append(xs)
            css01.append(cst)
        bias_sb = consts.tile([128, QW + KVW], F32, tag="bias", name="bias")
        nc.sync.dma_start(out=bias_sb, in_=bias_d[:, :])

        # resident weights, posted in e-order in ~512KB chunks
        wq_sb = wpool.tile([128, NE * QW], BF16, tag="wq", name="wq")
        wkv_sb = wpool.tile([128, NE * KVW], BF16, tag="wkv", name="wkv")
        for ch in range(8):
            c0, c1 = ch * 4 * QW, (ch + 1) * 4 * QW
            nc.sync.dma_start(out=wq_sb[:, c0:c1], in_=wq_d[:, c0:c1])
            if ch % 2 == 0:
                k0, k1 = ch * 4 * KVW, (ch + 2) * 4 * KVW
                nc.sync.dma_start(out=wkv_sb[:, k0:k1], in_=wkv_d[:, k0:k1])
        mask_sb = consts.tile([128, 896], F32, tag="mask", name="mask")
        nc.sync.dma_start(out=mask_sb, in_=mask_d[:, :])

        # out-proj weights last (not needed until phase C)
        wo_sb = wpool.tile([128, HP * EMB], BF16, tag="wo", name="wo")
        nc.sync.dma_start(out=wo_sb[:, 0:HP * EMB // 2], in_=wo_d[:, 0:HP * EMB // 2])
        nc.sync.dma_start(out=wo_sb[:, HP * EMB // 2:], in_=wo_d[:, HP * EMB // 2:])

        # resident activations
        qT = [resid.tile([128, T], BF16, tag=f"qT{h}", name=f"qT{h}") for h in range(HP)]
        kT = resid.tile([128, T], BF16, tag="kT", name="kT")
        vsb = [resid.tile([128, 128], BF16, tag=f"v{j}", name=f"v{j}") for j in range(NT)]
        ctxT = [resid.tile([128, T], BF16, tag=f"ctxT{h}", name=f"ctxT{h}") for h in range(HP)]

        # ---------------- Phase A: projections + rms + rope + transpose ----
        # q layout is half-split head-interleaved: cols [h*64+d'] hold head
        # h dims 0..63, cols [256+h*64+d'] hold dims 64..127.  RoPE for all
        # 4 heads is then 6 wide DVE ops instead of 24 narrow ones.
        def strip_epilogue(it, q_ps, kv_ps, cs):
            nc.vector.tensor_add(q_ps, q_ps, bias_sb[:, 0:QW])
            nc.vector.tensor_add(kv_ps, kv_ps, bias_sb[:, QW:QW + KVW])
            # rms stats: per-head two-piece views, one sqrt+recip for all 5
            q3d = q_ps.rearrange("p (two h d) -> p two h d", two=2, h=HP, d=64)
            sq5 = small.tile([128, 8], F32, tag="sq5", name="sq5")
            for b in range(HP + 1):
                src = q3d[:, :, b] if b < HP else kv_ps[:, 0:128]
                sqout = scratch.tile([128, 128], F32, tag="sqout", name="sqout")
                so = (sqout.rearrange("p (two d) -> p two d", two=2)
                      if b < HP else sqout)
                nc.scalar.activation(
                    out=so, in_=src,
                    func=mybir.ActivationFunctionType.Square,
                    accum_out=sq5[:, b:b + 1],
                )
            rstd5 = small.tile([128, 8], F32, tag="rstd5", name="rstd5")
            nc.scalar.activation(
                out=rstd5[:, 0:5], in_=sq5[:, 0:5],
                func=mybir.ActivationFunctionType.Sqrt,
                bias=eps_t, scale=1.0 / D,
            )
            nc.vector.reciprocal_approx_fast(out=rstd5[:, 0:5], in_=rstd5[:, 0:5])
            # q rope, all 4 heads at once: out1 = x1*c1 - x2*s1; out2 = x2*c2 + x1*s2
            x1, x2 = q_ps[:, 0:256], q_ps[:, 256:512]
            qrt = scratch.tile([128, 512], F32, tag="qrt", name="qrt")
            qm = scratch.tile([128, 256], F32, tag="qm", name="qm")
            nc.vector.tensor_mul(qrt[:, 0:256], x1, cs[:, 0:256])
            nc.vector.tensor_mul(qm, x2, cs[:, 512:768])
            nc.vector.tensor_sub(qrt[:, 0:256], qrt[:, 0:256], qm)
            nc.vector.tensor_mul(qrt[:, 256:512], x2, cs[:, 256:512])
            nc.vector.tensor_mul(qm, x1, cs[:, 768:1024])
            nc.vector.tensor_add(qrt[:, 256:512], qrt[:, 256:512], qm)
            qrt3d = qrt.rearrange("p (two h d) -> p two h d", two=2, h=HP, d=64)
            # k rope
            ksrc = kv_ps[:, 0:128]
            c_t, s_t = cs[:, 1024:1152], cs[:, 1152:1280]
            krt = scratch.tile([128, 128], F32, tag="krt", name="krt")
            km = scratch.tile([128, 64], F32, tag="km", name="km")
            nc.vector.tensor_mul(krt[:, 0:64], ksrc[:, 0:64], c_t[:, 0:64])
            nc.vector.tensor_mul(km, ksrc[:, 64:128], s_t[:, 0:64])
            nc.vector.tensor_sub(krt[:, 0:64], krt[:, 0:64], km)
            nc.vector.tensor_mul(krt[:, 64:128], ksrc[:, 64:128], c_t[:, 64:128])
            nc.vector.tensor_mul(km, ksrc[:, 0:64], s_t[:, 64:128])
            nc.vector.tensor_add(krt[:, 64:128], krt[:, 64:128], km)
            for b in range(HP + 1):  # 0..3 q heads, 4 = k
                rb = scratch.tile([128, 128], BF16, tag="rb", name="rb")
                if b < HP:
                    nc.vector.tensor_scalar_mul(
                        rb.rearrange("p (two d) -> p two d", two=2),
                        qrt3d[:, :, b], rstd5[:, b:b + 1])
                else:
                    nc.vector.tensor_scalar_mul(rb, krt, rstd5[:, b:b + 1])
                tp = psC.tile([128, 128], BF16, tag="d", name="tp")
                nc.tensor.transpose(tp, rb, ident)
                dst = qT[b] if b < HP else kT
                nc.scalar.copy(out=dst[:, it * 128:(it + 1) * 128], in_=tp)
            # v
            nc.scalar.copy(out=vsb[it], in_=kv_ps[:, 128:256])

        # strips 0+1 share one e-loop so the PE consumes freshly arriving
        # weight chunks at half rate during the initial weight download
        qkv01 = []
        for it in range(2):
            qkv01.append((psA.tile([128, QW], F32, tag="m", name=f"q_ps{it}"),
                          psB.tile([128, KVW], F32, tag="c", name=f"kv_ps{it}")))
        for e in range(NE):
            for it in range(2):
                xt = xstrips01[it][:, e * 128:(e + 1) * 128]
                q_ps, kv_ps = qkv01[it]
                nc.tensor.matmul(q_ps, xt, wq_sb[:, e * QW:(e + 1) * QW],
                                 start=(e == 0), stop=(e == NE - 1))
                nc.tensor.matmul(kv_ps, xt, wkv_sb[:, e * KVW:(e + 1) * KVW],
                                 start=(e == 0), stop=(e == NE - 1))
        strip_epilogue(0, qkv01[0][0], qkv01[0][1], css01[0])

        # epilogue for strip it-1 is emitted after strip it's matmuls so the
        # PE-side transposes have a full strip of slack behind the DVE chain
        pending = (1, qkv01[1][0], qkv01[1][1], css01[1])
        for it in range(2, NT):
            xstrip = xpool.tile([128, NE * 128], BF16, tag="xstrip",
                                name=f"xstrip{it}")
            r0, r1 = it * 128, (it + 1) * 128
            nc.sync.dma_start(out=xstrip[:, 0:2048], in_=xT_d[r0:r1, 0:2048])
            nc.sync.dma_start(out=xstrip[:, 2048:4096], in_=xT_d[r0:r1, 2048:4096])
            cs = cspool.tile([128, 1280], F32, tag="cs", name=f"cs{it}")
            nc.sync.dma_start(out=cs, in_=cs_d[r0:r1, :])

            q_ps = psA.tile([128, QW], F32, tag="m", name="q_ps")
            kv_ps = psB.tile([128, KVW], F32, tag="c", name="kv_ps")
            for e in range(NE):
                xt = xstrip[:, e * 128:(e + 1) * 128]
                nc.tensor.matmul(q_ps, xt, wq_sb[:, e * QW:(e + 1) * QW],
                                 start=(e == 0), stop=(e == NE - 1))
                nc.tensor.matmul(kv_ps, xt, wkv_sb[:, e * KVW:(e + 1) * KVW],
                                 start=(e == 0), stop=(e == NE - 1))
            strip_epilogue(*pending)
            pending = (it, q_ps, kv_ps, cs)
        strip_epilogue(*pending)

        # ---------------- Phase B + C interleaved per 512-query slice ------
        # C units (one (it, ot) out-proj group each) of slice si-1 are
        # interleaved into B(si)'s block pipeline so the PE stays fed
        # through B's ACT(exp)-bound stretches.
        LOOKAHEAD = 3

        def c_unit(it, ot, osb):
            o_ps = psA.tile([128, 512], F32, tag="m", name="o_ps")
            for hh in range(HP):
                nc.tensor.matmul(
                    o_ps,
                    ctxT[hh][:, it * 128:(it + 1) * 128],
                    wo_sb[:, hh * EMB + ot * 512:hh * EMB + (ot + 1) * 512],
                    start=(hh == 0), stop=(hh == HP - 1),
                )
            nc.scalar.copy(out=osb[:, ot * 512:(ot + 1) * 512], in_=o_ps)
            if ot == NO - 1:
                nc.sync.dma_start(
                    out=out_d[it * 128:(it + 1) * 128, :], in_=osb)

        def c_units_for(si):
            units = []
            for it in range(4 * si, 4 * si + 4):
                osb = opool.tile([128, EMB], BF16, tag="osb", name="osb")
                for ot in range(NO):
                    units.append((it, ot, osb))
            return units

        for si in range(T // 512):
            njb = 4 * si + 4
            blocks = [(h, jb) for h in range(HP) for jb in range(njb)]
            pending_c = c_units_for(si - 1) if si > 0 else []
            pace = max(1, (len(blocks) + len(pending_c) - 1) //
                       max(1, len(pending_c))) if pending_c else 0
            s_tiles = {}

            def emit_s(idx):
                h, jb = blocks[idx]
                kk = jb - 4 * si
                # masked-out query columns of diagonal blocks are skipped
                q0 = 128 * kk if kk > 0 else 0
                s_ps = psA.tile([128, 512], F32, tag="m", name="s_ps")
                nc.tensor.matmul(
                    s_ps[:, q0:512], kT[:, jb * 128:(jb + 1) * 128],
                    qT[h][:, si * 512 + q0:(si + 1) * 512],
                    start=True, stop=True,
                )
                if kk >= 0:  # diagonal (partially masked) block
                    nc.vector.tensor_add(
                        s_ps[:, q0:512], s_ps[:, q0:512],
                        mask_sb[:, 384:384 + 512 - q0])
                s_tiles[idx] = (s_ps, q0)

            for idx in range(min(LOOKAHEAD, len(blocks))):
                emit_s(idx)
            ctx_ps = pacc = None
            for i, (h, jb) in enumerate(blocks):
                s_ps, q0 = s_tiles.pop(i)
                p_t = ppool.tile([128, 512], BF16, tag="pt", name="pt")
                nc.scalar.activation(
                    out=p_t[:, q0:512], in_=s_ps[:, q0:512],
                    func=mybir.ActivationFunctionType.Exp,
                    scale=SM_SCALE,
                )
                if i + LOOKAHEAD < len(blocks):
                    emit_s(i + LOOKAHEAD)
                if jb == 0:
                    ctx_ps = psB.tile([128, 512], F32, tag="c", name="ctx_ps")
                    pacc = epool.tile([128, 512], F32, tag="pacc", name="pacc")
                    nc.vector.tensor_copy(out=pacc, in_=p_t)
                else:
                    nc.vector.tensor_add(pacc[:, q0:512], pacc[:, q0:512],
                                         p_t[:, q0:512])
                nc.tensor.matmul(ctx_ps[:, q0:512], vsb[jb], p_t[:, q0:512],
                                 start=(jb == 0), stop=(jb == njb - 1),
                                 skip_group_check=True)
                if pending_c and pace and i % pace == pace - 1:
                    c_unit(*pending_c.pop(0))
                if jb == njb - 1:
                    # denominator: one f32r matmul broadcasts the partition
                    # sum of the accumulated exp mass to all 128 partitions
                    den_ps = psC.tile([128, 512], F32, tag="d", name="den_ps")
                    nc.tensor.matmul(den_ps, ones_f32, pacc,
                                     start=True, stop=True)
                    rden = epool.tile([128, 512], F32, tag="rden", name="rden")
                    nc.vector.reciprocal_approx_fast(out=rden, in_=den_ps)
                    nc.vector.tensor_mul(
                        ctxT[h][:, si * 512:(si + 1) * 512], ctx_ps, rden)

            # leftover C units of si-1 not consumed by the pacing
            for u in pending_c:
                c_unit(*u)

        # final slice's out projection (no following B to interleave with)
        for u in c_units_for(T // 512 - 1):
            c_unit(*u)

    return nc


def _prep_inputs(x, mask, cos, sin, wq, bq, wk, bk, wv, bv, wo, q_scale, k_scale):
    x2 = np.asarray(x, dtype=np.float32).reshape(T, EMB)
    # strip layout: row (it*128 + p), col (eb*128 + t) holds x[it*128+t, eb*128+p]
    xTt = x2.reshape(NT, 128, NE, 128).transpose(0, 3, 2, 1)
    xTt = np.ascontiguousarray(xTt).reshape(NT * 128, NE * 128).astype(BF)

    qs = np.asarray(q_scale, dtype=np.float32)
    ks = np.asarray(k_scale, dtype=np.float32)
    qs_rot = np.concatenate([qs[64:], qs[:64]])
    ks_rot = np.concatenate([ks[64:], ks[:64]])
    cos = np.asarray(cos, dtype=np.float32)
    sin = np.asarray(sin, dtype=np.float32)
    cosq = cos * qs[None, :]
    sinq = sin * qs_rot[None, :]
    # q tables tiled 4-wide: [c1 x4 | c2 x4], matching half-split q layout
    cq4 = np.concatenate([np.tile(cosq[:, 0:64], (1, HP)),
                          np.tile(cosq[:, 64:128], (1, HP))], axis=1)
    sq4 = np.concatenate([np.tile(sinq[:, 0:64], (1, HP)),
                          np.tile(sinq[:, 64:128], (1, HP))], axis=1)
    cs = np.concatenate([cq4, sq4, cos * ks[None, :], sin * ks_rot[None, :]],
                        axis=1)
    cs = np.ascontiguousarray(cs, dtype=np.float32)

    def q_halfsplit(a):
        # permute last axis from [h][half][d'] to [half][h][d']
        return (a.reshape(*a.shape[:-1], HP, 2, 64)
                .swapaxes(-3, -2)
                .reshape(*a.shape))

    jj = np.arange(128)[:, None]
    cc = np.arange(896)[None, :]
    maskT = np.where(jj > cc - 384, NEG, 0.0).astype(np.float32)

    wq = np.asarray(wq, dtype=np.float32)
    wk = np.asarray(wk, dtype=np.float32)
    wv = np.asarray(wv, dtype=np.float32)
    wo = np.asarray(wo, dtype=np.float32)
    bq = np.asarray(bq, dtype=np.float32)
    bk = np.asarray(bk, dtype=np.float32)
    bv = np.asarray(bv, dtype=np.float32)

    in_maps = []
    for c in range(NCORES):
        # [p, e*QW + o] = wq[e*128 + p, c*QW + perm(o)]
        wq_c = q_halfsplit(wq[:, c * QW:(c + 1) * QW]).reshape(NE, 128, QW)
        wq_c = np.ascontiguousarray(wq_c.transpose(1, 0, 2)).reshape(128, NE * QW)
        wkv_c = np.concatenate(
            [wk[:, c * D:(c + 1) * D], wv[:, c * D:(c + 1) * D]], axis=1)
        wkv_c = wkv_c.reshape(NE, 128, KVW)
        wkv_c = np.ascontiguousarray(wkv_c.transpose(1, 0, 2)).reshape(128, NE * KVW)
        # [p, h*EMB + col] = wo[c*QW + h*128 + p, col]
        wo_c = wo[c * QW:(c + 1) * QW, :].reshape(HP, 128, EMB)
        wo_c = np.ascontiguousarray(wo_c.transpose(1, 0, 2)).reshape(128, HP * EMB)
        bias_c = np.broadcast_to(
            np.concatenate([q_halfsplit(bq[c * QW:(c + 1) * QW]),
                            bk[c * D:(c + 1) * D], bv[c * D:(c + 1) * D]]),
            (128, QW + KVW))
        in_maps.append({
            "xT": xTt,
            "wq": wq_c.astype(BF),
            "wkv": wkv_c.astype(BF),
            "wo": wo_c.astype(BF),
            "cs": cs,
            "maskT": maskT,
            "biasb": np.ascontiguousarray(bias_c, dtype=np.float32),
        })
    return in_maps


def _get_program():
    if "nc" not in _prog_cache:
        nc = _build_program()
        if not nc.is_finalized():
            nc.finalize()
        _prog_cache["nc"] = nc
    return _prog_cache["nc"]


def kernel(**inputs):
    in_maps = _prep_inputs(**inputs)
    nc = _get_program()
    res = run_bass_kernel_spmd(nc, in_maps, list(range(NCORES)))
    out = np.zeros((T, EMB), dtype=np.float32)
    for r in res.results:
        out += np.asarray(r["out"], dtype=np.float32)
    return out.reshape(1, T, EMB)


# revision 46
# speedup vs baseline: 1.3423x; 1.0000x over previous
"""GQA attention kernel for Trainium2, tensor-parallel over heads on 8 cores.

Problem: B=1, T=2048, EMB=4096, H=32 query heads, G=8 KV groups, D=128.
Reference: q/k/v projections -> per-head RMS norm (q,k) -> RoPE (q,k) ->
causal GQA attention -> out projection.

Sharding: core c owns query heads [4c, 4c+4) and KV group c.  Each core
computes a partial output for its heads; host sums the 8 partials (the
all-reduce of the module's TP scheme, done on host since full I/O is
required anyway).

Optimization history (709us baseline -> ~427us measured):
  - Phase B runs as one flat software pipeline per 512-query slice across
    all 4 heads: S matmuls emitted 3 blocks ahead of the exp->ctx chain so
    the PE never stalls on ACT latency and holds its max p-state clock.
  - Out-projection units of slice si-1 are interleaved into B(si)'s block
    pipeline, filling the PE during B's exp(ACT)-bound stretches and
    draining output DMA throughout attention.
  - Diagonal (causal) blocks narrowed to their unmasked query columns in
    S/mask/exp/ctx; softmax denominator = DVE-accumulated exp mass + one
    broadcast matmul (all-ones stationary) per group, off the PE hot path.
  - reciprocal_approx_fast (5x faster DVE reciprocal, 18-bit) for both
    softmax denominators and rms rstd (batched: one sqrt+recip per strip).
  - q stored half-split head-interleaved (all first-halves then second
    halves) so RoPE for 4 heads is 6 wide DVE ops instead of 24 narrow.
  - Each strip's rms/rope/transpose epilogue is emitted one strip late so
    the PE-side transposes have a strip of slack behind the DVE chain.
  - All weights pre-swizzled host-side to partition-major layouts, posted
    as ~512KB chunks in e-order; strips 0+1 share one e-loop so the PE
    consumes freshly arriving weight chunks at half rate during startup.
  - bf16 partial outputs (halves out-DMA; host sums partials in fp32).
"""

import numpy as np
import ml_dtypes
from contextlib import ExitStack

import concourse.bass as bass
import concourse.bacc as bacc
import concourse.mybir as mybir
from concourse.tile import TileContext
from concourse.bass_utils import run_bass_kernel_spmd
from concourse.masks import make_identity

EMB, H, G, D, T = 4096, 32, 8, 128, 2048
EPS = 1e-6
NCORES = 8
HP = H // NCORES          # 4 query heads per core
NT = T // 128             # 16 t-tiles
NE = EMB // 128           # 32 e-tiles
NO = EMB // 512           # 8 output column tiles
QW = HP * D               # 512 = q width per core
KVW = 2 * D               # 256 = k|v width per core
SM_SCALE = 1.0 / float(np.sqrt(D))
NEG = -1e9

F32 = mybir.dt.float32
BF16 = mybir.dt.bfloat16
BF = ml_dtypes.bfloat16

_prog_cache = {}


def _build_program():
    nc = bacc.Bacc()

    xT_d = nc.declare_dram_parameter("xT", [NT * 128, NE * 128], BF16, isOutput=False)
    wq_d = nc.declare_dram_parameter("wq", [128, NE * QW], BF16, isOutput=False)
    wkv_d = nc.declare_dram_parameter("wkv", [128, NE * KVW], BF16, isOutput=False)
    wo_d = nc.declare_dram_parameter("wo", [128, HP * EMB], BF16, isOutput=False)
    # [cosq x4 | sinq x4 | cosk | sink]; q tables tiled 4-wide to match the
    # half-split head-interleaved q layout (all first-halves, then second)
    cs_d = nc.declare_dram_parameter("cs", [NT * 128, 1280], F32, isOutput=False)
    mask_d = nc.declare_dram_parameter("maskT", [128, 896], F32, isOutput=False)
    bias_d = nc.declare_dram_parameter("biasb", [128, QW + KVW], F32, isOutput=False)
    out_d = nc.declare_dram_parameter("out", [T, EMB], BF16, isOutput=True)

    with TileContext(nc) as tc, ExitStack() as ctx:
        consts = ctx.enter_context(tc.tile_pool(name="consts", bufs=1))
        wpool = ctx.enter_context(tc.tile_pool(name="wpool", bufs=1))
        xpool = ctx.enter_context(tc.tile_pool(name="xpool", bufs=3))
        cspool = ctx.enter_context(tc.tile_pool(name="cspool", bufs=2))
        scratch = ctx.enter_context(tc.tile_pool(name="scratch", bufs=3))
        small = ctx.enter_context(tc.tile_pool(name="small", bufs=4))
        ppool = ctx.enter_context(tc.tile_pool(name="ppool", bufs=4))
        epool = ctx.enter_context(tc.tile_pool(name="epool", bufs=2))
        opool = ctx.enter_context(tc.tile_pool(name="opool", bufs=2))
        resid = ctx.enter_context(tc.tile_pool(name="resid", bufs=1))
        psA = ctx.enter_context(tc.tile_pool(name="psA", bufs=4, space="PSUM"))
        psB = ctx.enter_context(tc.tile_pool(name="psB", bufs=2, space="PSUM"))
        psC = ctx.enter_context(tc.tile_pool(name="psC", bufs=2, space="PSUM"))

        # engine-side constants (no DMA involved)
        ident = consts.tile([128, 128], BF16, tag="ident", name="ident")
        make_identity(nc, ident)
        ones_f32 = consts.tile([128, 128], F32, tag="ones_f32", name="ones_f32")
        nc.vector.memset(ones_f32, 1.0)
        eps_t = consts.tile([128, 1], F32, tag="eps", name="eps")
        nc.vector.memset(eps_t, EPS)

        # strips 0/1 inputs first so phase A can start while weights stream in
        xstrips01 = []
        css01 = []
        for it in range(2):
            xs = xpool.tile([128, NE * 128], BF16, tag="xstrip", name=f"xstrip{it}")
            r0, r1 = it * 128, (it + 1) * 128
            nc.sync.dma_start(out=xs[:, 0:2048], in_=xT_d[r0:r1, 0:2048])
            nc.sync.dma_start(out=xs[:, 2048:4096], in_=xT_d[r0:r1, 2048:4096])
            cst = cspool.tile([128, 1280], F32, tag="cs", name=f"cs{it}")
            nc.sync.dma_start(out=cst, in_=cs_d[r0:r1, :])
            xstrips01.append(xs)
            css01.append(cst)
        bias_sb = consts.tile([128, QW + KVW], F32, tag="bias", name="bias")
        nc.sync.dma_start(out=bias_sb, in_=bias_d[:, :])

        # resident weights, posted in e-order in ~512KB chunks
        wq_sb = wpool.tile([128, NE * QW], BF16, tag="wq", name="wq")
        wkv_sb = wpool.tile([128, NE * KVW], BF16, tag="wkv", name="wkv")
        for ch in range(8):
            c0, c1 = ch * 4 * QW, (ch + 1) * 4 * QW
            nc.sync.dma_start(out=wq_sb[:, c0:c1], in_=wq_d[:, c0:c1])
            if ch % 2 == 0:
                k0, k1 = ch * 4 * KVW, (ch + 2) * 4 * KVW
                nc.sync.dma_start(out=wkv_sb[:, k0:k1], in_=wkv_d[:, k0:k1])
        mask_sb = consts.tile([128, 896], F32, tag="mask", name="mask")
        nc.sync.dma_start(out=mask_sb, in_=mask_d[:, :])

        # out-proj weights last (not needed until phase C)
        wo_sb = wpool.tile([128, HP * EMB], BF16, tag="wo", name="wo")
        nc.sync.dma_start(out=wo_sb[:, 0:HP * EMB // 2], in_=wo_d[:, 0:HP * EMB // 2])
        nc.sync.dma_start(out=wo_sb[:, HP * EMB // 2:], in_=wo_d[:, HP * EMB // 2:])

        # resident activations
        qT = [resid.tile([128, T], BF16, tag=f"qT{h}", name=f"qT{h}") for h in range(HP)]
        kT = resid.tile([128, T], BF16, tag="kT", name="kT")
        vsb = [resid.tile([128, 128], BF16, tag=f"v{j}", name=f"v{j}") for j in range(NT)]
        ctxT = [resid.tile([128, T], BF16, tag=f"ctxT{h}", name=f"ctxT{h}") for h in range(HP)]

        # ---------------- Phase A: projections + rms + rope + transpose ----
        # q layout is half-split head-interleaved: cols [h*64+d'] hold head
        # h dims 0..63, cols [256+h*64+d'] hold dims 64..127.  RoPE for all
        # 4 heads is then 6 wide DVE ops instead of 24 narrow ones.
        def strip_epilogue(it, q_ps, kv_ps, cs):
            nc.vector.tensor_add(q_ps, q_ps, bias_sb[:, 0:QW])
            nc.vector.tensor_add(kv_ps, kv_ps, bias_sb[:, QW:QW + KVW])
            # rms stats: per-head two-piece views, one sqrt+recip for all 5
            q3d = q_ps.rearrange("p (two h d) -> p two h d", two=2, h=HP, d=64)
            sq5 = small.tile([128, 8], F32, tag="sq5", name="sq5")
            for b in range(HP + 1):
                src = q3d[:, :, b] if b < HP else kv_ps[:, 0:128]
                sqout = scratch.tile([128, 128], F32, tag="sqout", name="sqout")
                so = (sqout.rearrange("p (two d) -> p two d", two=2)
                      if b < HP else sqout)
                nc.scalar.activation(
                    out=so, in_=src,
                    func=mybir.ActivationFunctionType.Square,
                    accum_out=sq5[:, b:b + 1],
                )
            rstd5 = small.tile([128, 8], F32, tag="rstd5", name="rstd5")
            nc.scalar.activation(
                out=rstd5[:, 0:5], in_=sq5[:, 0:5],
                func=mybir.ActivationFunctionType.Sqrt,
                bias=eps_t, scale=1.0 / D,
            )
            nc.vector.reciprocal_approx_fast(out=rstd5[:, 0:5], in_=rstd5[:, 0:5])
            # q rope, all 4 heads at once: out1 = x1*c1 - x2*s1; out2 = x2*c2 + x1*s2
            x1, x2 = q_ps[:, 0:256], q_ps[:, 256:512]
            qrt = scratch.tile([128, 512], F32, tag="qrt", name="qrt")
            qm = scratch.tile([128, 256], F32, tag="qm", name="qm")
            nc.vector.tensor_mul(qrt[:, 0:256], x1, cs[:, 0:256])
            nc.vector.tensor_mul(qm, x2, cs[:, 512:768])
            nc.vector.tensor_sub(qrt[:, 0:256], qrt[:, 0:256], qm)
            nc.vector.tensor_mul(qrt[:, 256:512], x2, cs[:, 256:512])
            nc.vector.tensor_mul(qm, x1, cs[:, 768:1024])
            nc.vector.tensor_add(qrt[:, 256:512], qrt[:, 256:512], qm)
            qrt3d = qrt.rearrange("p (two h d) -> p two h d", two=2, h=HP, d=64)
            # k rope
            ksrc = kv_ps[:, 0:128]
            c_t, s_t = cs[:, 1024:1152], cs[:, 1152:1280]
            krt = scratch.tile([128, 128], F32, tag="krt", name="krt")
            km = scratch.tile([128, 64], F32, tag="km", name="km")
            nc.vector.tensor_mul(krt[:, 0:64], ksrc[:, 0:64], c_t[:, 0:64])
            nc.vector.tensor_mul(km, ksrc[:, 64:128], s_t[:, 0:64])
            nc.vector.tensor_sub(krt[:, 0:64], krt[:, 0:64], km)
            nc.vector.tensor_mul(krt[:, 64:128], ksrc[:, 64:128], c_t[:, 64:128])
            nc.vector.tensor_mul(km, ksrc[:, 0:64], s_t[:, 64:128])
            nc.vector.tensor_add(krt[:, 64:128], krt[:, 64:128], km)
            for b in range(HP + 1):  # 0..3 q heads, 4 = k
                rb = scratch.tile([128, 128], BF16, tag="rb", name="rb")
                if b < HP:
                    nc.vector.tensor_scalar_mul(
                        rb.rearrange("p (two d) -> p two d", two=2),
                        qrt3d[:, :, b], rstd5[:, b:b + 1])
                else:
                    nc.vector.tensor_scalar_mul(rb, krt, rstd5[:, b:b + 1])
                tp = psC.tile([128, 128], BF16, tag="d", name="tp")
                nc.tensor.transpose(tp, rb, ident)
                dst = qT[b] if b < HP else kT
                nc.scalar.copy(out=dst[:, it * 128:(it + 1) * 128], in_=tp)
            # v
            nc.scalar.copy(out=vsb[it], in_=kv_ps[:, 128:256])

        # strips 0+1 share one e-loop so the PE consumes freshly arriving
        # weight chunks at half rate during the initial weight download
        qkv01 = []
        for it in range(2):
            qkv01.append((psA.tile([128, QW], F32, tag="m", name=f"q_ps{it}"),
                          psB.tile([128, KVW], F32, tag="c", name=f"kv_ps{it}")))
        for e in range(NE):
            for it in range(2):
                xt = xstrips01[it][:, e * 128:(e + 1) * 128]
                q_ps, kv_ps = qkv01[it]
                nc.tensor.matmul(q_ps, xt, wq_sb[:, e * QW:(e + 1) * QW],
                                 start=(e == 0), stop=(e == NE - 1))
                nc.tensor.matmul(kv_ps, xt, wkv_sb[:, e * KVW:(e + 1) * KVW],
                                 start=(e == 0), stop=(e == NE - 1))
        strip_epilogue(0, qkv01[0][0], qkv01[0][1], css01[0])

        # epilogue for strip it-1 is emitted after strip it's matmuls so the
        # PE-side transposes have a full strip of slack behind the DVE chain
        pending = (1, qkv01[1][0], qkv01[1][1], css01[1])
        for it in range(2, NT):
            xstrip = xpool.tile([128, NE * 128], BF16, tag="xstrip",
                                name=f"xstrip{it}")
            r0, r1 = it * 128, (it + 1) * 128
            nc.sync.dma_start(out=xstrip[:, 0:2048], in_=xT_d[r0:r1, 0:2048])
            nc.sync.dma_start(out=xstrip[:, 2048:4096], in_=xT_d[r0:r1, 2048:4096])
            cs = cspool.tile([128, 1280], F32, tag="cs", name=f"cs{it}")
            nc.sync.dma_start(out=cs, in_=cs_d[r0:r1, :])

            q_ps = psA.tile([128, QW], F32, tag="m", name="q_ps")
            kv_ps = psB.tile([128, KVW], F32, tag="c", name="kv_ps")
            for e in range(NE):
                xt = xstrip[:, e * 128:(e + 1) * 128]
                nc.tensor.matmul(q_ps, xt, wq_sb[:, e * QW:(e + 1) * QW],
                                 start=(e == 0), stop=(e == NE - 1))
                nc.tensor.matmul(kv_ps, xt, wkv_sb[:, e * KVW:(e + 1) * KVW],
                                 start=(e == 0), stop=(e == NE - 1))
            strip_epilogue(*pending)
            pending = (it, q_ps, kv_ps, cs)
        strip_epilogue(*pending)

        # ---------------- Phase B + C interleaved per 512-query slice ------
        # C units (one (it, ot) out-proj group each) of slice si-1 are
        # interleaved into B(si)'s block pipeline so the PE stays fed
        # through B's ACT(exp)-bound stretches.
        LOOKAHEAD = 3

        def c_unit(it, ot, osb):
            o_ps = psA.tile([128, 512], F32, tag="m", name="o_ps")
            for hh in range(HP):
                nc.tensor.matmul(
                    o_ps,
                    ctxT[hh][:, it * 128:(it + 1) * 128],
                    wo_sb[:, hh * EMB + ot * 512:hh * EMB + (ot + 1) * 512],
                    start=(hh == 0), stop=(hh == HP - 1),
                )
            nc.scalar.copy(out=osb[:, ot * 512:(ot + 1) * 512], in_=o_ps)
            if ot == NO - 1:
                nc.sync.dma_start(
                    out=out_d[it * 128:(it + 1) * 128, :], in_=osb)

        def c_units_for(si):
            units = []
            for it in range(4 * si, 4 * si + 4):
                osb = opool.tile([128, EMB], BF16, tag="osb", name="osb")
                for ot in range(NO):
                    units.append((it, ot, osb))
            return units

        for si in range(T // 512):
            njb = 4 * si + 4
            blocks = [(h, jb) for h in range(HP) for jb in range(njb)]
            pending_c = c_units_for(si - 1) if si > 0 else []
            pace = max(1, (len(blocks) + len(pending_c) - 1) //
                       max(1, len(pending_c))) if pending_c else 0
            s_tiles = {}

            def emit_s(idx):
                h, jb = blocks[idx]
                kk = jb - 4 * si
                # masked-out query columns of diagonal blocks are skipped
                q0 = 128 * kk if kk > 0 else 0
                s_ps = psA.tile([128, 512], F32, tag="m", name="s_ps")
                nc.tensor.matmul(
                    s_ps[:, q0:512], kT[:, jb * 128:(jb + 1) * 128],
                    qT[h][:, si * 512 + q0:(si + 1) * 512],
                    start=True, stop=True,
                )
                if kk >= 0:  # diagonal (partially masked) block
                    nc.vector.tensor_add(
                        s_ps[:, q0:512], s_ps[:, q0:512],
                        mask_sb[:, 384:384 + 512 - q0])
                s_tiles[idx] = (s_ps, q0)

            for idx in range(min(LOOKAHEAD, len(blocks))):
                emit_s(idx)
            ctx_ps = pacc = None
            for i, (h, jb) in enumerate(blocks):
                s_ps, q0 = s_tiles.pop(i)
                p_t = ppool.tile([128, 512], BF16, tag="pt", name="pt")
                nc.scalar.activation(
                    out=p_t[:, q0:512], in_=s_ps[:, q0:512],
                    func=mybir.ActivationFunctionType.Exp,
                    scale=SM_SCALE,
                )
                if i + LOOKAHEAD < len(blocks):
                    emit_s(i + LOOKAHEAD)
                if jb == 0:
                    ctx_ps = psB.tile([128, 512], F32, tag="c", name="ctx_ps")
                    pacc = epool.tile([128, 512], F32, tag="pacc", name="pacc")
                    nc.vector.tensor_copy(out=pacc, in_=p_t)
                else:
                    nc.vector.tensor_add(pacc[:, q0:512], pacc[:, q0:512],
                                         p_t[:, q0:512])
                nc.tensor.matmul(ctx_ps[:, q0:512], vsb[jb], p_t[:, q0:512],
                                 start=(jb == 0), stop=(jb == njb - 1),
                                 skip_group_check=True)
                if pending_c and pace and i % pace == pace - 1:
                    c_unit(*pending_c.pop(0))
                if jb == njb - 1:
                    # denominator: one f32r matmul broadcasts the partition
                    # sum of the accumulated exp mass to all 128 partitions
                    den_ps = psC.tile([128, 512], F32, tag="d", name="den_ps")
                    nc.tensor.matmul(den_ps, ones_f32, pacc,
                                     start=True, stop=True)
                    rden = epool.tile([128, 512], F32, tag="rden", name="rden")
                    nc.vector.reciprocal_approx_fast(out=rden, in_=den_ps)
                    nc.vector.tensor_mul(
                        ctxT[h][:, si * 512:(si + 1) * 512], ctx_ps, rden)

            # leftover C units of si-1 not consumed by the pacing
            for u in pending_c:
                c_unit(*u)

        # final slice's out projection (no following B to interleave with)
        for u in c_units_for(T // 512 - 1):
            c_unit(*u)

    return nc


def _prep_inputs(x, mask, cos, sin, wq, bq, wk, bk, wv, bv, wo, q_scale, k_scale):
    x2 = np.asarray(x, dtype=np.float32).reshape(T, EMB)
    # strip layout: row (it*128 + p), col (eb*128 + t) holds x[it*128+t, eb*128+p]
    xTt = x2.reshape(NT, 128, NE, 128).transpose(0, 3, 2, 1)
    xTt = np.ascontiguousarray(xTt).reshape(NT * 128, NE * 128).astype(BF)

    qs = np.asarray(q_scale, dtype=np.float32)
    ks = np.asarray(k_scale, dtype=np.float32)
    qs_rot = np.concatenate([qs[64:], qs[:64]])
    ks_rot = np.concatenate([ks[64:], ks[:64]])
    cos = np.asarray(cos, dtype=np.float32)
    sin = np.asarray(sin, dtype=np.float32)
    cosq = cos * qs[None, :]
    sinq = sin * qs_rot[None, :]
    # q tables tiled 4-wide: [c1 x4 | c2 x4], matching half-split q layout
    cq4 = np.concatenate([np.tile(cosq[:, 0:64], (1, HP)),
                          np.tile(cosq[:, 64:128], (1, HP))], axis=1)
    sq4 = np.concatenate([np.tile(sinq[:, 0:64], (1, HP)),
                          np.tile(sinq[:, 64:128], (1, HP))], axis=1)
    cs = np.concatenate([cq4, sq4, cos * ks[None, :], sin * ks_rot[None, :]],
                        axis=1)
    cs = np.ascontiguousarray(cs, dtype=np.float32)

    def q_halfsplit(a):
        # permute last axis from [h][half][d'] to [half][h][d']
        return (a.reshape(*a.shape[:-1], HP, 2, 64)
                .swapaxes(-3, -2)
                .reshape(*a.shape))

    jj = np.arange(128)[:, None]
    cc = np.arange(896)[None, :]
    maskT = np.where(jj > cc - 384, NEG, 0.0).astype(np.float32)

    wq = np.asarray(wq, dtype=np.float32)
    wk = np.asarray(wk, dtype=np.float32)
    wv = np.asarray(wv, dtype=np.float32)
    wo = np.asarray(wo, dtype=np.float32)
    bq = np.asarray(bq, dtype=np.float32)
    bk = np.asarray(bk, dtype=np.float32)
    bv = np.asarray(bv, dtype=np.float32)

    in_maps = []
    for c in range(NCORES):
        # [p, e*QW + o] = wq[e*128 + p, c*QW + perm(o)]
        wq_c = q_halfsplit(wq[:, c * QW:(c + 1) * QW]).reshape(NE, 128, QW)
        wq_c = np.ascontiguousarray(wq_c.transpose(1, 0, 2)).reshape(128, NE * QW)
        wkv_c = np.concatenate(
            [wk[:, c * D:(c + 1) * D], wv[:, c * D:(c + 1) * D]], axis=1)
        wkv_c = wkv_c.reshape(NE, 128, KVW)
        wkv_c = np.ascontiguousarray(wkv_c.transpose(1, 0, 2)).reshape(128, NE * KVW)
        # [p, h*EMB + col] = wo[c*QW + h*128 + p, col]
        wo_c = wo[c * QW:(c + 1) * QW, :].reshape(HP, 128, EMB)
        wo_c = np.ascontiguousarray(wo_c.transpose(1, 0, 2)).reshape(128, HP * EMB)
        bias_c = np.broadcast_to(
            np.concatenate([q_halfsplit(bq[c * QW:(c + 1) * QW]),
                            bk[c * D:(c + 1) * D], bv[c * D:(c + 1) * D]]),
            (128, QW + KVW))
        in_maps.append({
            "xT": xTt,
            "wq": wq_c.astype(BF),
            "wkv": wkv_c.astype(BF),
            "wo": wo_c.astype(BF),
            "cs": cs,
            "maskT": maskT,
            "biasb": np.ascontiguousarray(bias_c, dtype=np.float32),
        })
    return in_maps


def _get_program():
    if "nc" not in _prog_cache:
        nc = _build_program()
        if not nc.is_finalized():
            nc.finalize()
        _prog_cache["nc"] = nc
    return _prog_cache["nc"]


def kernel(**inputs):
    in_maps = _prep_inputs(**inputs)
    nc = _get_program()
    res = run_bass_kernel_spmd(nc, in_maps, list(range(NCORES)))
    out = np.zeros((T, EMB), dtype=np.float32)
    for r in res.results:
        out += np.asarray(r["out"], dtype=np.float32)
    return out.reshape(1, T, EMB)
